# revision 54
# baseline (speedup 1.0000x reference)
r"""Circulant layer kernel for Trainium2 (8 NeuronCores).

Math: reference computes mv1 + mv2 where
  mv1 = batch_circulant(b) @ d,  mv2 = batch_circulant(d) @ b,
with d = des @ K, b = body @ K.  Both are the circular convolution of d and b
(circular convolution is commutative), so  out = 2 * circconv(d, b).

circconv via DFT:  out = 2 * Re(IDFT(DFT(d) * DFT(b))).
DFT/IDFT are realized as dense matmuls with host-generated constant
cos/sin matrices (input-independent constants).

Sharding: real-input DFTs are conjugate-symmetric, so only frequencies
0..512 are needed; each of the 8 cores owns 64 of them (core 0 also
carries f=512 in a 65th slot; its imaginary part is identically zero so
the complex pointwise product needs no special casing).
Per core c:
  KC_c   = K @ CC_c            (1024k x 130s)   fused projection+forward DFT
  DT_c   = KC_c^T @ des^T      (130s x 128b)    \  shares stationary weights
  BT_c   = KC_c^T @ body^T     (130s x 128b)    /
  PT_c   = complex-mult(DT_c, BT_c)             (130s x 128b)  on VectorE
  part_c = (PT_c^T @ G_c)                       (128b x 1024)  inverse DFT
Host sums the 8 partials (unshard).

Schedule: the fixed framework overhead (prologue ~8.4us, per-semaphore
teardown ~9.5us, 316 EVENT_SEMAPHOREs) is invariant (a 1-matmul kernel
spans 22.4us), so only the body is compressible.  K^T ships COLUMN-split
(by kb output block) so the kb-outer stage-1 chains start as soon as
their own columns land (~13.4us) instead of after the whole K (~17.7us).
All DMA rows stay >= 8KB (shorter rows are descriptor-rate-bound: 20KB
rows stream at 407 GB/s, 2.5KB rows at ~300).  Three input transfers on
one serial SP queue (each extra transfer adds ~1us semaphore-release
skew to its gate).  Stage-2 is bundled after the chains (dbt lands last)
and the pointwise stage reads the stage-2 PSUM directly.
"""

import numpy as np

import concourse.bass as bass
import concourse.mybir as mybir
import concourse.tile as tile
from concourse.bass_utils import run_bass_kernel_spmd
from concourse.tile_rust import add_dep_helper

B = 128        # batch
D_IN = 1024    # input feature dim (contraction k)
N = 1024       # output feature dim (conv length j) == #frequencies
N_CORES = 8
FPC = 64            # frequencies per core (conjugate symmetry: only 0..512
                    # are needed; each core owns 64, core 0 also carries 512)
R = FPC + 1         # slots per r/i block (64 freqs + the f=512/pad slot)
S = 2 * R           # freq slots per core: [0:R]=real(cos), [R:2R]=imag(-sin)

F32 = mybir.dt.float32
F32R = mybir.dt.float32r
BF16 = mybir.dt.bfloat16

# Matmul operand precision: "bf16" (fastest; ~5e-3 rel err), "f32r"
# (single-pass TF32-like; ~3e-4), "f32" (two-pass full fp32; ~7e-7).
import os as _os
MM_PREC = _os.environ.get("CIRC_MM_PREC", "bf16")
MM_DT = {"bf16": BF16, "f32r": F32R, "f32": F32}[MM_PREC]


def _np_in(a):
    """Cast to the matmul precision; bf16 data is shipped packed in fp32
    words (DMA is element-rate-bound: 2-byte elements run at half rate)."""
    import ml_dtypes
    a = np.ascontiguousarray(np.asarray(a, dtype=np.float32))
    if MM_PREC != "bf16":
        return a
    bf = np.ascontiguousarray(a.astype(ml_dtypes.bfloat16))
    return bf.view(np.uint8).reshape(a.shape[0], -1).view(np.float32)

# Number of fp32 transport words per logical input element.
PACK = 2 if MM_PREC == "bf16" else 1
# Transport dtype: bf16 ships packed in fp32 words; f32/f32r ship natively
# (the fp32r verifier requires the producing DMA to be f32r-typed).
TR_DT = F32 if MM_PREC == "bf16" else MM_DT

# Stashed by kernel() for test harnesses that want profiling info.
LAST_RESULT = None

_nc_cache = {}

JC = N // 128      # 8 chunks over j (contraction of KC stage)
KB = D_IN // 128   # 8 blocks over k (output partitions of KC stage)
SB = 2             # r and i slot blocks (R rows each)
KB0 = 4            # kb blocks in the first ktcc transfer (with CC)
WARM = 22          # PE warmup matmuls: cover [8.4us .. ktcc0 release ~14.8us]
                   # with no PE gap, else the HAM clock drops to half speed

# Words per j-chunk row segment of each transfer.
W0 = (S + KB0 * 128) // PACK        # [cc | kt kb0..KB0-1]
W1 = ((KB - KB0) * 128) // PACK     # [kt kbKB0..7]


def _build_nc():
    """Build the (single-program) Bass module run on all 8 cores."""
    nc = bass.Bass(target_bir_lowering=True)

    # Three input transfers, one serial SP chain, all rows >= 8KB:
    #   ktcc0[p] = per j-chunk: [CC row | K^T columns kb0..KB0-1]
    #   ktcc1[p] = per j-chunk: [K^T columns kb3..7]
    #   aux[p]   = [des^T|body^T k-chunks | G s-chunks]
    ktcc0_q = nc.declare_dram_parameter("ktcc0", [128, JC * W0], TR_DT, False)
    ktcc1_q = nc.declare_dram_parameter("ktcc1", [128, JC * W1], TR_DT, False)
    aux_q = nc.declare_dram_parameter("aux", [128, (2 * B * KB + SB * N) // PACK],
                                      TR_DT, False)
    out = nc.declare_dram_parameter("out", [B, N], F32, isOutput=True)
    warm_scratch = nc.dram_tensor("warm_scratch", [1, 4], F32)

    with tile.TileContext(nc) as tc:
        with (
            tc.tile_pool(name="main", bufs=1) as pool,
            tc.tile_pool(name="psum", bufs=1, space="PSUM") as pp,
        ):
            # ---- inputs -> SBUF ----
            ktcc0_sb = pool.tile([128, JC, W0], TR_DT, tag="ktcc0", name="ktcc0")
            ktcc1_sb = pool.tile([128, JC, W1], TR_DT, tag="ktcc1", name="ktcc1")
            aux_sb = pool.tile([128, (2 * B * KB + SB * N) // PACK], TR_DT,
                               tag="aux", name="aux")
            in_dmas = [
                nc.sync.dma_start(ktcc0_sb[:], ktcc0_q[:, :]),
                nc.sync.dma_start(ktcc1_sb[:], ktcc1_q[:, :]),
                nc.sync.dma_start(aux_sb[:], aux_q[:, :]),
            ]
            v0 = ktcc0_sb.bitcast(MM_DT)   # [128, JC, 2*W0]
            v1 = ktcc1_sb.bitcast(MM_DT)
            va = aux_sb.bitcast(MM_DT)
            cc_sb = [v0[:, j, :S] for j in range(JC)]

            def kt_block(j, kb):
                if kb < KB0:
                    return v0[:, j, S + kb * 128:S + (kb + 1) * 128]
                kb -= KB0
                return v1[:, j, kb * 128:(kb + 1) * 128]

            dbt_sl = [va[:, kb * 2 * B:(kb + 1) * 2 * B] for kb in range(KB)]
            g_sb = [va[:, 2 * B * KB + s * N:2 * B * KB + (s + 1) * N]
                    for s in range(SB)]

            # ---- PE warmup: keep the HAM clock un-throttled while DMAs
            # stream in, so the real matmuls all run at 2.4 GHz. Dead-code
            # proofed by a tiny gpsimd DMA of the result to scratch DRAM.
            wz = pool.tile([128, 384], BF16, tag="wz", name="wz")
            nc.gpsimd.memset(wz[:], 0.0)
            wps = pp.tile([128, 256], F32, tag="wps", name="wps")
            for w in range(WARM):
                nc.tensor.matmul(wps[:], wz[:, :128], wz[:, 128:384],
                                 start=True, stop=True)
            wsb = pool.tile([128, 4], F32, tag="wsb", name="wsb")
            nc.vector.tensor_copy(wsb[:], wps[:, :4])
            warm_dma = nc.gpsimd.dma_start(warm_scratch[:, :], wsb[:1, :])

            # ---- stage 1: kb-outer chains; kb0..KB0-1 start when ktcc0 lands ----
            # KC[k, s] = sum_j KT[j, k] * CC[j, s]
            kc_sb = [pool.tile([128, S], MM_DT, tag=f"kc{kb}", name=f"kc{kb}")
                     for kb in range(KB)]
            for kb in range(KB):
                ps = pp.tile([128, S], F32, tag="kcp", name=f"kcp{kb}", bufs=2)
                for j in range(JC):
                    nc.tensor.matmul(ps[:], kt_block(j, kb), cc_sb[j][:],
                                     start=(j == 0), stop=(j == JC - 1))
                nc.vector.tensor_copy(kc_sb[kb][:], ps[:])

            # ---- stage 2 (bundled: dbt is in the last transfer) ----
            db_ps = [pp.tile([R, 2 * B], F32, tag=f"dbp{sb}", name=f"dbp{sb}")
                     for sb in range(SB)]
            for kb in range(KB):
                for sb in range(SB):
                    nc.tensor.matmul(db_ps[sb][:],
                                     kc_sb[kb][:, sb * R:(sb + 1) * R],
                                     dbt_sl[kb],
                                     start=(kb == 0), stop=(kb == KB - 1))
            # filler matmuls: keep the PE active through the pointwise stage
            # so the HAM clock stays up for stage 4 (kcp ring: the drain
            # edge is the kb6 cast, a single wait).
            fill = pp.tile([128, S], F32, tag="kcp", name="fill", bufs=2)
            for w in range(7):
                nc.tensor.matmul(fill[:], kc_sb[0][:, :128], kc_sb[1][:],
                                 start=True, stop=True)

            # ---- stage 3: complex pointwise multiply (on freq partitions) ----
            # Reads the stage-2 PSUM directly; vector ops may read only ONE
            # operand from PSUM, so the B-halves stage through SBUF.
            # t01 = [Dr*Br, Dr*Bi], t23 = [Di*Bi, Di*Br]
            # Pr = t01[0] - t23[0],  Pi = t01[1] + t23[1]
            t01 = pool.tile([R, 2, B], F32, tag="t01", name="t01")
            t23 = pool.tile([R, 2, B], F32, tag="t23", name="t23")
            pt = pool.tile([R, 2, B], MM_DT, tag="pt", name="pt")
            bb = pool.tile([R, 2, B], F32, tag="bb", name="bb")
            nc.vector.tensor_copy(bb[:, 0, :], db_ps[0][:, B:])
            nc.vector.tensor_copy(bb[:, 1, :], db_ps[1][:, B:])
            dr_b = db_ps[0][:, :B][:, None, :].to_broadcast((R, 2, B))
            di_b = db_ps[1][:, :B][:, None, :].to_broadcast((R, 2, B))
            nc.vector.tensor_mul(t01[:], dr_b, bb[:])
            nc.vector.tensor_mul(t23[:], di_b, bb[:, ::-1, :])
            nc.vector.tensor_sub(pt[:, 0, :], t01[:, 0, :], t23[:, 0, :])
            nc.vector.tensor_add(pt[:, 1, :], t01[:, 1, :], t23[:, 1, :])
            pt_sb = [pt[:, sb, :] for sb in range(SB)]

            # ---- stage 4: part = PT^T @ G ----
            # Four quarter-pipelines: each quarter's copy and store overlap
            # the later quarters' matmuls.  Stores: 3 on the SP queue (3
            # inputs + 3 stores = 6 transfers, within its ~7 limit), the
            # last on ACT (its only data-dep transfer); copies alternate
            # Vector / ACT so they pair up in parallel.
            out_sb = pool.tile([128, N], F32, tag="outsb", name="outsb")
            last_mm = None
            out_cps = []
            stores = []
            Q = N // 4
            for h in range(4):
                o_ps = pp.tile([128, Q], F32, tag="op", name=f"op{h}", bufs=2)
                for sb in range(SB):
                    last_mm = nc.tensor.matmul(
                        o_ps[:],
                        pt_sb[sb],
                        g_sb[sb][:R, h * Q:(h + 1) * Q],
                        start=(sb == 0),
                        stop=(sb == SB - 1),
                    )
                cp_eng = nc.vector.tensor_copy if h % 2 == 0 else nc.scalar.copy
                out_cps.append(cp_eng(out_sb[:, h * Q:(h + 1) * Q], o_ps[:]))
                st_eng = nc.scalar if h == 3 else nc.sync
                stores.append(st_eng.dma_start(out[:, h * Q:(h + 1) * Q],
                                               out_sb[:, h * Q:(h + 1) * Q]))

            # TileContext's exit emits one tail Drain waiting on every
            # outstanding semaphore; walrus caps instructions at ONE sync
            # wait.  Pre-absorb every tick into SP's clock with a chain of
            # single-wait drains so the tail drain needs none.
            prev = None
            for dep in [*in_dmas, warm_dma, *stores, last_mm, *out_cps]:
                dr = nc.sync.drain(fusable=False)
                add_dep_helper(dr.ins, dep.ins, sync=True,
                               reason="tail: absorb tick into SP clock")
                if prev is not None:
                    add_dep_helper(dr.ins, prev.ins, sync=False,
                                   reason="tail: keep drain chain ordered")
                prev = dr

    return nc


def _dft_constants():
    """Per-core CC (N x S) and G rows (128 x 2N, rows 0..R-1 live).

    Conjugate symmetry: the half-spectrum reconstruction is
      out[k] = (2/N) [P_0 + (-1)^k P_512 + sum_{f=1}^{511} 2 Re(P_f e^{i2pifk/N})]
    (the leading 2 is the out = 2*circconv factor).  Weights w_f = 2 except
    w_0 = w_512 = 1, absorbed into G.  Core 0's extra slot (index FPC)
    carries f=512; other cores pad it with zeros.
    """
    jj = np.arange(N, dtype=np.float64)
    k = np.arange(N, dtype=np.float64)
    ccs, gs = [], []
    for c in range(N_CORES):
        f = np.arange(c * FPC, (c + 1) * FPC, dtype=np.float64)
        ang = 2.0 * np.pi * np.outer(jj, f) / N          # (j, f)
        extra_r = (np.cos(np.pi * jj)[:, None] if c == 0
                   else np.zeros((N, 1)))
        cc = np.concatenate(
            [np.cos(ang), extra_r, -np.sin(ang), np.zeros((N, 1))], axis=1)
        w = np.where(f == 0, 1.0, 2.0)
        angT = 2.0 * np.pi * np.outer(f, k) / N          # (f, k)
        gr = (2.0 / N) * w[:, None] * np.cos(angT)
        gi = -(2.0 / N) * w[:, None] * np.sin(angT)
        g = np.zeros((128, 2 * N))
        g[:FPC, :N] = gr
        if c == 0:
            g[FPC, :N] = (2.0 / N) * np.cos(np.pi * k)
        g[:FPC, N:] = gi
        ccs.append(np.ascontiguousarray(cc, dtype=np.float32))
        gs.append(np.ascontiguousarray(g, dtype=np.float32))
    return ccs, gs


def _partition_pack(a):
    """(R, W) with R = n*128 -> (128, n*W): row p = concat of chunk rows p."""
    r, w = a.shape
    n = r // 128
    return np.ascontiguousarray(
        a.reshape(n, 128, w).transpose(1, 0, 2).reshape(128, n * w))


def kernel(des, body, kernel):
    global LAST_RESULT
    K = np.asarray(kernel, dtype=np.float32)
    kt_np = K.T  # (j, k)
    dbt_np = _partition_pack(_np_in(np.concatenate(
        [np.asarray(des, dtype=np.float32).T, np.asarray(body, dtype=np.float32).T],
        axis=1,
    )))  # (k, 2B) packed
    ccs, gs = _dft_constants()
    split = KB0 * 128
    ktcc0s = [
        _partition_pack(_np_in(np.concatenate([ccs[c], kt_np[:, :split]], axis=1)))
        for c in range(N_CORES)
    ]
    ktcc1 = _partition_pack(_np_in(kt_np[:, split:]))
    auxs = [
        np.ascontiguousarray(np.concatenate([dbt_np, _np_in(gs[c])], axis=1))
        for c in range(N_CORES)
    ]

    if "nc" not in _nc_cache:
        _nc_cache["nc"] = _build_nc()
    nc = _nc_cache["nc"]

    in_maps = [
        {"ktcc0": ktcc0s[c], "ktcc1": ktcc1, "aux": auxs[c]}
        for c in range(N_CORES)
    ]
    res = run_bass_kernel_spmd(nc, in_maps, list(range(N_CORES)))
    LAST_RESULT = res
    out = np.zeros((B, N), dtype=np.float32)
    for r in res.results:
        out += r["out"]
    return out


# revision 55
# speedup vs baseline: 1.0207x; 1.0207x over previous
r"""Circulant layer kernel for Trainium2 (8 NeuronCores).

Math: reference computes mv1 + mv2 where
  mv1 = batch_circulant(b) @ d,  mv2 = batch_circulant(d) @ b,
with d = des @ K, b = body @ K.  Both are the circular convolution of d and b
(circular convolution is commutative), so  out = 2 * circconv(d, b).

circconv via DFT:  out = 2 * Re(IDFT(DFT(d) * DFT(b))).
DFT/IDFT are realized as dense matmuls with host-generated constant
cos/sin matrices (input-independent constants).

Sharding: real-input DFTs are conjugate-symmetric, so only frequencies
0..512 are needed; each of the 8 cores owns 64 of them (core 0 also
carries f=512 in a 65th slot; its imaginary part is identically zero so
the complex pointwise product needs no special casing).
Per core c:
  KC_c   = K @ CC_c            (1024k x 130s)   fused projection+forward DFT
  DT_c   = KC_c^T @ des^T      (130s x 128b)    \  shares stationary weights
  BT_c   = KC_c^T @ body^T     (130s x 128b)    /
  PT_c   = complex-mult(DT_c, BT_c)             (130s x 128b)  on VectorE
  part_c = (PT_c^T @ G_c)                       (128b x 1024)  inverse DFT
Host sums the 8 partials (unshard).

Schedule: the fixed framework overhead (prologue ~8.4us, per-semaphore
teardown ~9.5us, 316 EVENT_SEMAPHOREs) is invariant (a 1-matmul kernel
spans 22.4us), so only the body is compressible.  K^T ships COLUMN-split
(by kb output block) so the kb-outer stage-1 chains start as soon as
their own columns land (~13.4us) instead of after the whole K (~17.7us).
All DMA rows stay >= 8KB (shorter rows are descriptor-rate-bound: 20KB
rows stream at 407 GB/s, 2.5KB rows at ~300).  Three input transfers on
one serial SP queue (each extra transfer adds ~1us semaphore-release
skew to its gate).  Stage-2 is bundled after the chains (dbt lands last)
and the pointwise stage reads the stage-2 PSUM directly.
"""

import numpy as np

import concourse.bass as bass
import concourse.mybir as mybir
import concourse.tile as tile
from concourse.bass_utils import run_bass_kernel_spmd
from concourse.tile_rust import add_dep_helper

B = 128        # batch
D_IN = 1024    # input feature dim (contraction k)
N = 1024       # output feature dim (conv length j) == #frequencies
N_CORES = 8
FPC = 64            # frequencies per core (conjugate symmetry: only 0..512
                    # are needed; each core owns 64, core 0 also carries 512)
R = FPC + 1         # slots per r/i block (64 freqs + the f=512/pad slot)
S = 2 * R           # freq slots per core: [0:R]=real(cos), [R:2R]=imag(-sin)

F32 = mybir.dt.float32
F32R = mybir.dt.float32r
BF16 = mybir.dt.bfloat16

# Matmul operand precision: "bf16" (fastest; ~5e-3 rel err), "f32r"
# (single-pass TF32-like; ~3e-4), "f32" (two-pass full fp32; ~7e-7).
import os as _os
MM_PREC = _os.environ.get("CIRC_MM_PREC", "bf16")
MM_DT = {"bf16": BF16, "f32r": F32R, "f32": F32}[MM_PREC]


def _np_in(a):
    """Cast to the matmul precision; bf16 data is shipped packed in fp32
    words (DMA is element-rate-bound: 2-byte elements run at half rate)."""
    import ml_dtypes
    a = np.ascontiguousarray(np.asarray(a, dtype=np.float32))
    if MM_PREC != "bf16":
        return a
    bf = np.ascontiguousarray(a.astype(ml_dtypes.bfloat16))
    return bf.view(np.uint8).reshape(a.shape[0], -1).view(np.float32)

# Number of fp32 transport words per logical input element.
PACK = 2 if MM_PREC == "bf16" else 1
# Transport dtype: bf16 ships packed in fp32 words; f32/f32r ship natively
# (the fp32r verifier requires the producing DMA to be f32r-typed).
TR_DT = F32 if MM_PREC == "bf16" else MM_DT

# Stashed by kernel() for test harnesses that want profiling info.
LAST_RESULT = None

_nc_cache = {}

JC = N // 128      # 8 chunks over j (contraction of KC stage)
KB = D_IN // 128   # 8 blocks over k (output partitions of KC stage)
SB = 2             # r and i slot blocks (R rows each)
KB0 = 4            # kb blocks in the first ktcc transfer (with CC)
WARM = 22          # PE warmup matmuls: cover [8.4us .. ktcc0 release ~14.8us]
                   # with no PE gap, else the HAM clock drops to half speed

# Words per j-chunk row segment of each transfer.
W0 = (S + KB0 * 128) // PACK        # [cc | kt kb0..KB0-1]
W1 = ((KB - KB0) * 128) // PACK     # [kt kbKB0..7]


def _build_nc():
    """Build the (single-program) Bass module run on all 8 cores."""
    nc = bass.Bass(target_bir_lowering=True)

    # Three input transfers, one serial SP chain, all rows >= 8KB:
    #   ktcc0[p] = per j-chunk: [CC row | K^T columns kb0..KB0-1]
    #   ktcc1[p] = per j-chunk: [K^T columns kb3..7]
    #   aux[p]   = [des^T|body^T k-chunks | G s-chunks]
    ktcc0_q = nc.declare_dram_parameter("ktcc0", [128, JC * W0], TR_DT, False)
    ktcc1_q = nc.declare_dram_parameter("ktcc1", [128, JC * W1], TR_DT, False)
    aux_q = nc.declare_dram_parameter("aux", [128, (2 * B * KB + SB * N) // PACK],
                                      TR_DT, False)
    out = nc.declare_dram_parameter("out", [B, N], F32, isOutput=True)
    warm_scratch = nc.dram_tensor("warm_scratch", [1, 4], F32)

    with tile.TileContext(nc) as tc:
        with (
            tc.tile_pool(name="main", bufs=1) as pool,
            tc.tile_pool(name="psum", bufs=1, space="PSUM") as pp,
        ):
            # ---- inputs -> SBUF ----
            ktcc0_sb = pool.tile([128, JC, W0], TR_DT, tag="ktcc0", name="ktcc0")
            ktcc1_sb = pool.tile([128, JC, W1], TR_DT, tag="ktcc1", name="ktcc1")
            aux_sb = pool.tile([128, (2 * B * KB + SB * N) // PACK], TR_DT,
                               tag="aux", name="aux")
            in_dmas = [
                nc.sync.dma_start(ktcc0_sb[:], ktcc0_q[:, :]),
                nc.sync.dma_start(ktcc1_sb[:], ktcc1_q[:, :]),
                nc.sync.dma_start(aux_sb[:], aux_q[:, :]),
            ]
            v0 = ktcc0_sb.bitcast(MM_DT)   # [128, JC, 2*W0]
            v1 = ktcc1_sb.bitcast(MM_DT)
            va = aux_sb.bitcast(MM_DT)
            cc_sb = [v0[:, j, :S] for j in range(JC)]

            def kt_block(j, kb):
                if kb < KB0:
                    return v0[:, j, S + kb * 128:S + (kb + 1) * 128]
                kb -= KB0
                return v1[:, j, kb * 128:(kb + 1) * 128]

            dbt_sl = [va[:, kb * 2 * B:(kb + 1) * 2 * B] for kb in range(KB)]
            g_sb = [va[:, 2 * B * KB + s * N:2 * B * KB + (s + 1) * N]
                    for s in range(SB)]

            # ---- PE warmup: keep the HAM clock un-throttled while DMAs
            # stream in, so the real matmuls all run at 2.4 GHz. Dead-code
            # proofed by a tiny gpsimd DMA of the result to scratch DRAM.
            wz = pool.tile([128, 384], BF16, tag="wz", name="wz")
            nc.gpsimd.memset(wz[:], 0.0)
            wps = pp.tile([128, 256], F32, tag="wps", name="wps")
            for w in range(WARM):
                nc.tensor.matmul(wps[:], wz[:, :128], wz[:, 128:384],
                                 start=True, stop=True)
            wsb = pool.tile([128, 4], F32, tag="wsb", name="wsb")
            nc.vector.tensor_copy(wsb[:], wps[:, :4])
            warm_dma = nc.gpsimd.dma_start(warm_scratch[:, :], wsb[:1, :])

            # ---- stage 1: kb-outer chains; kb0..KB0-1 start when ktcc0 lands ----
            # KC[k, s] = sum_j KT[j, k] * CC[j, s]
            kc_sb = [pool.tile([128, S], MM_DT, tag=f"kc{kb}", name=f"kc{kb}")
                     for kb in range(KB)]
            for kb in range(KB):
                ps = pp.tile([128, S], F32, tag="kcp", name=f"kcp{kb}", bufs=2)
                for j in range(JC):
                    nc.tensor.matmul(ps[:], kt_block(j, kb), cc_sb[j][:],
                                     start=(j == 0), stop=(j == JC - 1))
                nc.vector.tensor_copy(kc_sb[kb][:], ps[:])

            # ---- stage 2 (bundled: dbt is in the last transfer) ----
            db_ps = [pp.tile([R, 2 * B], F32, tag=f"dbp{sb}", name=f"dbp{sb}")
                     for sb in range(SB)]
            for kb in range(KB):
                for sb in range(SB):
                    nc.tensor.matmul(db_ps[sb][:],
                                     kc_sb[kb][:, sb * R:(sb + 1) * R],
                                     dbt_sl[kb],
                                     start=(kb == 0), stop=(kb == KB - 1))
            # filler matmuls: keep the PE active through the pointwise stage
            # so the HAM clock stays up for stage 4 (kcp ring: the drain
            # edge is the kb6 cast, a single wait).
            fill = pp.tile([128, S], F32, tag="kcp", name="fill", bufs=2)
            for w in range(7):
                nc.tensor.matmul(fill[:], kc_sb[0][:, :128], kc_sb[1][:],
                                 start=True, stop=True)

            # ---- stage 3: complex pointwise multiply (on freq partitions) ----
            # Reads the stage-2 PSUM directly; vector ops may read only ONE
            # operand from PSUM, so the B-halves stage through SBUF.
            # t01 = [Dr*Br, Dr*Bi], t23 = [Di*Bi, Di*Br]
            # Pr = t01[0] - t23[0],  Pi = t01[1] + t23[1]
            t01 = pool.tile([R, 2, B], F32, tag="t01", name="t01")
            t23 = pool.tile([R, 2, B], F32, tag="t23", name="t23")
            pt = pool.tile([R, 2, B], MM_DT, tag="pt", name="pt")
            bb = pool.tile([R, 2, B], F32, tag="bb", name="bb")
            nc.vector.tensor_copy(bb[:, 0, :], db_ps[0][:, B:])
            nc.vector.tensor_copy(bb[:, 1, :], db_ps[1][:, B:])
            dr_b = db_ps[0][:, :B][:, None, :].to_broadcast((R, 2, B))
            di_b = db_ps[1][:, :B][:, None, :].to_broadcast((R, 2, B))
            nc.vector.tensor_mul(t01[:], dr_b, bb[:])
            nc.vector.tensor_mul(t23[:], di_b, bb[:, ::-1, :])
            nc.vector.tensor_sub(pt[:, 0, :], t01[:, 0, :], t23[:, 0, :])
            nc.vector.tensor_add(pt[:, 1, :], t01[:, 1, :], t23[:, 1, :])
            pt_sb = [pt[:, sb, :] for sb in range(SB)]

            # ---- stage 4: part = PT^T @ G ----
            # Four quarter-pipelines: each quarter's copy and store overlap
            # the later quarters' matmuls.  Stores: 3 on the SP queue (3
            # inputs + 3 stores = 6 transfers, within its ~7 limit), the
            # last on ACT (its only data-dep transfer); copies alternate
            # Vector / ACT so they pair up in parallel.
            out_sb = pool.tile([128, N], F32, tag="outsb", name="outsb")
            last_mm = None
            out_cps = []
            stores = []
            Q = N // 4
            for h in range(4):
                o_ps = pp.tile([128, Q], F32, tag="op", name=f"op{h}", bufs=3)
                for sb in range(SB):
                    last_mm = nc.tensor.matmul(
                        o_ps[:],
                        pt_sb[sb],
                        g_sb[sb][:R, h * Q:(h + 1) * Q],
                        start=(sb == 0),
                        stop=(sb == SB - 1),
                    )
                cp_eng = nc.vector.tensor_copy if h % 2 == 0 else nc.scalar.copy
                out_cps.append(cp_eng(out_sb[:, h * Q:(h + 1) * Q], o_ps[:]))
                st_eng = nc.scalar if h == 3 else nc.sync
                stores.append(st_eng.dma_start(out[:, h * Q:(h + 1) * Q],
                                               out_sb[:, h * Q:(h + 1) * Q]))

            # TileContext's exit emits one tail Drain waiting on every
            # outstanding semaphore; walrus caps instructions at ONE sync
            # wait.  Pre-absorb every tick into SP's clock with a chain of
            # single-wait drains so the tail drain needs none.
            prev = None
            for dep in [*in_dmas, warm_dma, *stores, last_mm, *out_cps]:
                dr = nc.sync.drain(fusable=False)
                add_dep_helper(dr.ins, dep.ins, sync=True,
                               reason="tail: absorb tick into SP clock")
                if prev is not None:
                    add_dep_helper(dr.ins, prev.ins, sync=False,
                                   reason="tail: keep drain chain ordered")
                prev = dr

    return nc


def _dft_constants():
    """Per-core CC (N x S) and G rows (128 x 2N, rows 0..R-1 live).

    Conjugate symmetry: the half-spectrum reconstruction is
      out[k] = (2/N) [P_0 + (-1)^k P_512 + sum_{f=1}^{511} 2 Re(P_f e^{i2pifk/N})]
    (the leading 2 is the out = 2*circconv factor).  Weights w_f = 2 except
    w_0 = w_512 = 1, absorbed into G.  Core 0's extra slot (index FPC)
    carries f=512; other cores pad it with zeros.
    """
    jj = np.arange(N, dtype=np.float64)
    k = np.arange(N, dtype=np.float64)
    ccs, gs = [], []
    for c in range(N_CORES):
        f = np.arange(c * FPC, (c + 1) * FPC, dtype=np.float64)
        ang = 2.0 * np.pi * np.outer(jj, f) / N          # (j, f)
        extra_r = (np.cos(np.pi * jj)[:, None] if c == 0
                   else np.zeros((N, 1)))
        cc = np.concatenate(
            [np.cos(ang), extra_r, -np.sin(ang), np.zeros((N, 1))], axis=1)
        w = np.where(f == 0, 1.0, 2.0)
        angT = 2.0 * np.pi * np.outer(f, k) / N          # (f, k)
        gr = (2.0 / N) * w[:, None] * np.cos(angT)
        gi = -(2.0 / N) * w[:, None] * np.sin(angT)
        g = np.zeros((128, 2 * N))
        g[:FPC, :N] = gr
        if c == 0:
            g[FPC, :N] = (2.0 / N) * np.cos(np.pi * k)
        g[:FPC, N:] = gi
        ccs.append(np.ascontiguousarray(cc, dtype=np.float32))
        gs.append(np.ascontiguousarray(g, dtype=np.float32))
    return ccs, gs


def _partition_pack(a):
    """(R, W) with R = n*128 -> (128, n*W): row p = concat of chunk rows p."""
    r, w = a.shape
    n = r // 128
    return np.ascontiguousarray(
        a.reshape(n, 128, w).transpose(1, 0, 2).reshape(128, n * w))


def kernel(des, body, kernel):
    global LAST_RESULT
    K = np.asarray(kernel, dtype=np.float32)
    kt_np = K.T  # (j, k)
    dbt_np = _partition_pack(_np_in(np.concatenate(
        [np.asarray(des, dtype=np.float32).T, np.asarray(body, dtype=np.float32).T],
        axis=1,
    )))  # (k, 2B) packed
    ccs, gs = _dft_constants()
    split = KB0 * 128
    ktcc0s = [
        _partition_pack(_np_in(np.concatenate([ccs[c], kt_np[:, :split]], axis=1)))
        for c in range(N_CORES)
    ]
    ktcc1 = _partition_pack(_np_in(kt_np[:, split:]))
    auxs = [
        np.ascontiguousarray(np.concatenate([dbt_np, _np_in(gs[c])], axis=1))
        for c in range(N_CORES)
    ]

    if "nc" not in _nc_cache:
        _nc_cache["nc"] = _build_nc()
    nc = _nc_cache["nc"]

    in_maps = [
        {"ktcc0": ktcc0s[c], "ktcc1": ktcc1, "aux": auxs[c]}
        for c in range(N_CORES)
    ]
    res = run_bass_kernel_spmd(nc, in_maps, list(range(N_CORES)))
    LAST_RESULT = res
    out = np.zeros((B, N), dtype=np.float32)
    for r in res.results:
        out += r["out"]
    return out
